# revision 4
# baseline (speedup 1.0000x reference)
"""CenterLoss kernel for Trainium2 (8 NeuronCores, data-parallel over batch).

reference:
    gathered = centers[labels]            # [B, D] gather from [V, D]
    loss = sum((feat - gathered)**2) / B / 2

Sharding: feat/labels split along batch across 8 cores; centers replicated
(each core only reads the rows its labels hit, via indirect DMA gather).
Each core emits a partial sum-of-squares scalar; host sums the 8 partials
and applies the / B / 2 normalization.
"""

import numpy as np

import concourse.bass as bass
import concourse.bacc as bacc
import concourse.tile as tile
from concourse import mybir
from concourse.bass_utils import run_bass_kernel_spmd

NUM_CLASSES = 100000
D = 256
B = 16384
N_CORES = 8
B_SHARD = B // N_CORES  # 2048
P = 128
T = B_SHARD // P  # 16 rows per partition
N_CHUNKS = 4
TC = T // N_CHUNKS  # tiles (rows/partition) per chunk
CD = TC * D  # free-dim elements per chunk

_CACHE = {}


def build_nc():
    # Bacc (not plain Bass): its compile() runs generate_event_semaphores(),
    # which splits Tile's multi-wait instructions into the 1-wait-per-inst
    # form the walrus backend requires.
    nc = bacc.Bacc("TRN2", target_bir_lowering=False)
    # feat shard viewed [P, T*D]: partition p holds rows [p*T, (p+1)*T).
    feat = nc.declare_dram_parameter("feat", [P, T * D], mybir.dt.float32, isOutput=False)
    # labels shard viewed [P, T]: labels[p, t] pairs with feat row (p, t).
    labels = nc.declare_dram_parameter("labels", [P, T], mybir.dt.int32, isOutput=False)
    centers = nc.declare_dram_parameter(
        "centers", [NUM_CLASSES, D], mybir.dt.float32, isOutput=False
    )
    out = nc.declare_dram_parameter("out", [1, 1], mybir.dt.float32, isOutput=True)

    with tile.TileContext(nc) as tc:
        with (
            tc.tile_pool(name="sbuf", bufs=2) as pool,
            tc.tile_pool(name="consts", bufs=1) as cpool,
            tc.tile_pool(name="psum", bufs=1, space="PSUM") as psum_pool,
        ):
            labels_sb = cpool.tile([P, T], mybir.dt.int32, tag="labels")
            nc.sync.dma_start(out=labels_sb[:], in_=labels[:, :])

            ones = cpool.tile([P, 1], mybir.dt.float32, tag="ones")
            nc.vector.memset(ones[:], 1.0)

            acc = cpool.tile([P, N_CHUNKS], mybir.dt.float32, tag="acc")

            for c in range(N_CHUNKS):
                feat_sb = pool.tile([P, CD], mybir.dt.float32, tag="feat")
                nc.sync.dma_start(
                    out=feat_sb[:], in_=feat[:, c * CD : (c + 1) * CD]
                )
                gath_sb = pool.tile([P, CD], mybir.dt.float32, tag="gath")
                nc.gpsimd.indirect_dma_start(
                    out=gath_sb[:],
                    out_offset=None,
                    in_=centers[:],
                    in_offset=bass.IndirectOffsetOnAxis(
                        ap=labels_sb[:, c * TC : (c + 1) * TC], axis=0
                    ),
                )
                diff = pool.tile([P, CD], mybir.dt.float32, tag="diff")
                nc.vector.tensor_sub(out=diff[:], in0=feat_sb[:], in1=gath_sb[:])
                sq = pool.tile([P, CD], mybir.dt.float32, tag="sq")
                nc.scalar.activation(
                    sq[:],
                    diff[:],
                    mybir.ActivationFunctionType.Square,
                    accum_out=acc[:, c : c + 1],
                )

            red = cpool.tile([P, 1], mybir.dt.float32, tag="red")
            nc.vector.reduce_sum(red[:], acc[:], axis=mybir.AxisListType.X)
            res_psum = psum_pool.tile([1, 1], mybir.dt.float32)
            nc.tensor.matmul(
                out=res_psum[:], lhsT=ones[:], rhs=red[:], start=True, stop=True
            )
            out_sb = cpool.tile([1, 1], mybir.dt.float32, tag="out")
            nc.vector.tensor_copy(out=out_sb[:], in_=res_psum[:])
            nc.sync.dma_start(out=out[:, :], in_=out_sb[:])

    nc.compile()
    return nc


def _get_nc():
    if "nc" not in _CACHE:
        _CACHE["nc"] = build_nc()
    return _CACHE["nc"]


def kernel(feat, labels, centers):
    feat = np.ascontiguousarray(np.asarray(feat, dtype=np.float32))
    labels = np.asarray(labels)
    centers = np.ascontiguousarray(np.asarray(centers, dtype=np.float32))
    assert feat.shape == (B, D) and labels.shape == (B,)
    assert centers.shape == (NUM_CLASSES, D)

    labels_i32 = labels.astype(np.int32)

    nc = _get_nc()
    in_maps = []
    for c in range(N_CORES):
        lo, hi = c * B_SHARD, (c + 1) * B_SHARD
        in_maps.append(
            {
                "feat": feat[lo:hi].reshape(P, T * D),
                "labels": labels_i32[lo:hi].reshape(P, T),
                "centers": centers,
            }
        )
    res = run_bass_kernel_spmd(nc, in_maps, list(range(N_CORES)))
    total = sum(float(r["out"][0, 0]) for r in res.results)
    return np.float32(total / B / 2.0)
